# revision 6
# baseline (speedup 1.0000x reference)
"""AdaptiveChebConv (K=3) distributed Bass kernel for 8 TRN2 NeuronCores.

Data-parallel over batch: B=16 -> 2 batches per core. adj/Theta replicated.

Per-core algorithm (per local batch b; N=1024, F=O=64, T=12):
  A  = adj * attn_b                      (DVE elementwise, bf16)
  Z1 = A^T X                             (PE bf16; X natural [n,(f,t)])
  Z2 = A^T Z1                            (PE bf16)
  T[j] = transpose_t(X|Z1|Z2)            (PE transposes -> [f,(t,n)] bf16)
  out[n,o,t] = relu(sum_j Theta_j^T T_j) (PE bf16, K=64 accum x3, ACT relu)
"""
import sys

if "/opt/trn_rl_repo" not in sys.path:
    sys.path.insert(0, "/opt/trn_rl_repo")

import numpy as np
from contextlib import ExitStack

import concourse.bass as bass
import concourse.tile as tile
from concourse import bacc, mybir
from concourse.bass_utils import run_bass_kernel_spmd

N_CORES = 8
B, N, F, T, K, O = 16, 1024, 64, 12, 3, 64
BL = B // N_CORES          # local batches per core = 2
NT = N // 128              # n-tiles = 8
FT = F * T                 # 768
OT = O * T                 # 768
XSPLIT = 4                 # x DMA split (2 n-tiles per chunk)

F32 = mybir.dt.float32
BF16 = mybir.dt.bfloat16

_NC = None


class Ctx:
    pass


def _alloc_xz(cx, name):
    """4 tiles of 2 n-tiles each from the shared 8-slot xz tag; returns
    (tiles, slicer)."""
    tiles = [
        cx.xz_pool.tile([128, 2 * FT], BF16, tag="xz", bufs=8, name=f"{name}_{i}")
        for i in range(XSPLIT)
    ]

    def sl(mt, lo, size):
        return tiles[mt // 2][:, (mt % 2) * FT + lo: (mt % 2) * FT + lo + size]

    return tiles, sl


def _load_x(cx, b):
    """DMA x[b] (bf16 cast) into 4 tiles; returns slicer."""
    tiles, sl = _alloc_xz(cx, f"X{b}")
    for xs in range(XSPLIT):
        cx.nc.gpsimd.dma_start(
            tiles[xs][:],
            cx.x_ap[b, xs * 256:(xs + 1) * 256].rearrange(
                "(nt p) f t -> p nt (f t)", p=128
            ),
        )
    return sl


def _emit_tp_pack(cx, src3, j, nt, g, b):
    """One pack: 4 transposes (t = 4g..4g+3) -> psum -> T tile copy."""
    nc = cx.nc
    pt = cx.tp.tile([64, 512], BF16, tag="tp", name="pt")
    for ti in range(4):
        t = 4 * g + ti
        nc.tensor.transpose(
            pt[:, ti * 128:(ti + 1) * 128], src3[:, t, :], cx.ident_t[:]
        )
    eng_v = (nt + g) % 2 == 0
    dst = cx.T_t[(b, j, nt)][:, g * 512:(g + 1) * 512]
    if eng_v:
        nc.vector.tensor_copy(dst, pt[:])
    else:
        nc.scalar.activation(dst, pt[:], mybir.ActivationFunctionType.Copy)


def _emit_theta_group(cx, b, nt):
    """Theta contraction + relu + out DMA for one n-tile."""
    nc = cx.nc
    o_tile = cx.out_pool.tile([128, OT], F32, tag="out", name="o_tile")
    for (t0, tn) in ((0, 8), (8, 4)):
        pq = cx.qp.tile([128, 512], F32, tag="qp", name="pq")
        for ts in range(tn):
            t = t0 + ts
            for j in range(3):
                nc.tensor.matmul(
                    pq[:, ts * 64:(ts + 1) * 64],
                    cx.T_t[(b, j, nt)][:, t * 128:(t + 1) * 128],
                    cx.theta_t[:, j * 64:(j + 1) * 64],
                    start=(j == 0),
                    stop=(j == 2),
                )
        dst = o_tile[:].rearrange("p (o t) -> p t o", t=T)[:, t0:t0 + tn, :]
        src = pq[:, 0:tn * 64].rearrange("p (t o) -> p t o", o=64)
        nc.scalar.activation(dst, src, mybir.ActivationFunctionType.Relu)
    nc.sync.dma_start(
        cx.out_ap[b, nt * 128:(nt + 1) * 128, :, :].rearrange("p o t -> p (o t)"),
        o_tile[:],
    )


def _emit_A(cx, b):
    """A(b) = adj * attn[b], bf16 [128, NT*1024]."""
    nc = cx.nc
    A = cx.a_pool.tile([128, NT * 1024], BF16, tag="A", name=f"A{b}")
    attn_s = cx.scr_pool.tile(
        [128, NT * 1024], BF16, tag="attnscr", name=f"attn{b}"
    )
    nc.gpsimd.dma_start(
        attn_s[:], cx.attn_ap[b].rearrange("(mt p) n -> p mt n", p=128)
    )
    for mt in range(NT):
        sl = slice(mt * 1024, (mt + 1) * 1024)
        nc.vector.tensor_mul(A[:, sl], attn_s[:, sl], cx.adjbf_t[:, sl])
    return A


def _emit_big_matmul(cx, A, dst_fn, rhs_fn, tp_work):
    """dst = A^T rhs (16 psum groups); interleave tp_work packs with MMs."""
    nc = cx.nc
    wi = 0
    for nt in range(NT):
        for ch in range(2):
            pz = cx.zp.tile([128, 384], F32, tag="zp", name="pz")
            for mt in range(NT):
                nc.tensor.matmul(
                    pz[:],
                    A[:, mt * 1024 + nt * 128: mt * 1024 + (nt + 1) * 128],
                    rhs_fn(mt, ch * 384, 384),
                    start=(mt == 0),
                    stop=(mt == NT - 1),
                )
                if tp_work and wi < len(tp_work) and mt % 3 == 2:
                    tp_work[wi]()
                    wi += 1
            nc.vector.tensor_copy(dst_fn(nt, ch * 384, 384), pz[:])
    while tp_work and wi < len(tp_work):
        tp_work[wi]()
        wi += 1


def _emit_batch_main(cx, b, x_sl):
    nc = cx.nc
    A = _emit_A(cx, b)

    Z1 = cx.z_pool.tile([128, NT * FT], BF16, tag="Z1", name=f"Z1_{b}")

    def z1_sl(mt, lo, sz):
        return Z1[:, mt * FT + lo: mt * FT + lo + sz]

    _emit_big_matmul(cx, A, z1_sl, x_sl, tp_work=None)

    # Z2 matmuls with Z1-transposes interleaved between MMs
    z1tp = []
    for nt in range(NT):
        src3 = Z1[:, nt * FT:(nt + 1) * FT].rearrange("p (f t) -> p t f", t=T)
        for g in range(3):
            z1tp.append(
                (lambda s=src3, n=nt, gg=g: _emit_tp_pack(cx, s, 1, n, gg, b))
            )
    _, z2_sl = _alloc_xz(cx, f"Z2_{b}")
    _emit_big_matmul(cx, A, z2_sl, z1_sl, tp_work=z1tp)

    # Z2 transposes staggered with Theta groups of the previous n-tile
    for nt in range(NT):
        src3 = z2_sl(nt, 0, FT).rearrange("p (f t) -> p t f", t=T)
        for g in range(3):
            _emit_tp_pack(cx, src3, 2, nt, g, b)
        if nt > 0:
            _emit_theta_group(cx, b, nt - 1)
    _emit_theta_group(cx, b, NT - 1)


def _build():
    nc = bacc.Bacc("TRN2", target_bir_lowering=False, debug=False)
    cx = Ctx()
    cx.nc = nc
    cx.x_ap = nc.dram_tensor("x", [BL, N, F, T], F32, kind="ExternalInput").ap()
    cx.attn_ap = nc.dram_tensor(
        "spatial_attention", [BL, N, N], F32, kind="ExternalInput"
    ).ap()
    cx.adj_ap = nc.dram_tensor("adj", [N, N], F32, kind="ExternalInput").ap()
    cx.theta_ap = nc.dram_tensor("Theta", [K, F, O], F32, kind="ExternalInput").ap()
    cx.ident_ap = nc.dram_tensor("ident", [128, 128], F32, kind="ExternalInput").ap()
    cx.out_ap = nc.dram_tensor("out", [BL, N, O, T], F32, kind="ExternalOutput").ap()

    with tile.TileContext(nc) as tc, ExitStack() as ctx:
        cx.a_pool = ctx.enter_context(tc.tile_pool(name="apool", bufs=2))
        cx.xz_pool = ctx.enter_context(tc.tile_pool(name="xz", bufs=1))
        cx.z_pool = ctx.enter_context(tc.tile_pool(name="zpool", bufs=1))
        cx.t_pool = ctx.enter_context(tc.tile_pool(name="tpool", bufs=1))
        cx.scr_pool = ctx.enter_context(tc.tile_pool(name="scr", bufs=1))
        cx.out_pool = ctx.enter_context(tc.tile_pool(name="outp", bufs=3))
        const_pool = ctx.enter_context(tc.tile_pool(name="const", bufs=1))
        cx.zp = ctx.enter_context(tc.tile_pool(name="zp", bufs=3, space="PSUM"))
        cx.tp = ctx.enter_context(tc.tile_pool(name="tp", bufs=3, space="PSUM"))
        cx.qp = ctx.enter_context(tc.tile_pool(name="qp", bufs=2, space="PSUM"))

        cx.ident_t = const_pool.tile([128, 128], BF16, tag="ident")
        nc.gpsimd.dma_start(cx.ident_t[:], cx.ident_ap[:])
        cx.theta_t = const_pool.tile([64, K * O], BF16, tag="theta")
        nc.gpsimd.dma_start(cx.theta_t[:], cx.theta_ap.rearrange("k f o -> f k o"))
        cx.adjbf_t = const_pool.tile([128, NT * 1024], BF16, tag="adjbf")
        nc.gpsimd.dma_start(
            cx.adjbf_t[:], cx.adj_ap.rearrange("(mt p) n -> p mt n", p=128)
        )

        # T tiles: (batch, j, nt) -> [64, T*128]; X_T (j=0) per batch; j=1,2
        # single-buffered by tag across batches.
        cx.T_t = {}
        for b in range(BL):
            for j in range(3):
                for nt in range(NT):
                    tag = f"T{j}_{nt}" if j > 0 else f"T0_{nt}_{b}"
                    cx.T_t[(b, j, nt)] = cx.t_pool.tile(
                        [64, T * 128], BF16, tag=tag, name=f"T{b}{j}{nt}"
                    )

        # Both batches: x loads + X transposes up-front (PE filler during
        # the attn/adj DMA phase).
        x_sls = []
        for b in range(BL):
            x_sls.append(_load_x(cx, b))
        for b in range(BL):
            for nt in range(NT):
                src3 = x_sls[b](nt, 0, FT).rearrange("p (f t) -> p t f", t=T)
                for g in range(3):
                    _emit_tp_pack(cx, src3, 0, nt, g, b)

        for b in range(BL):
            _emit_batch_main(cx, b, x_sls[b])

    nc.compile()
    return nc


def kernel(**inputs):
    global _NC
    if _NC is None:
        _NC = _build()
    nc = _NC

    x = np.ascontiguousarray(inputs["x"], dtype=np.float32)
    attn = np.ascontiguousarray(inputs["spatial_attention"], dtype=np.float32)
    adj = np.ascontiguousarray(inputs["adj"], dtype=np.float32)
    theta = np.ascontiguousarray(inputs["Theta"], dtype=np.float32)
    ident = np.eye(128, dtype=np.float32)

    in_maps = []
    for i in range(N_CORES):
        s = slice(i * BL, (i + 1) * BL)
        in_maps.append(
            {
                "x": x[s],
                "spatial_attention": attn[s],
                "adj": adj,
                "Theta": theta,
                "ident": ident,
            }
        )
    res = run_bass_kernel_spmd(nc, in_maps, core_ids=list(range(N_CORES)))
    out = np.concatenate([res.results[i]["out"] for i in range(N_CORES)], axis=0)
    return out


# revision 7
# speedup vs baseline: 1.1798x; 1.1798x over previous
"""AdaptiveChebConv (K=3) distributed Bass kernel for 8 TRN2 NeuronCores.

Data-parallel over batch: B=16 -> 2 batches per core. adj/Theta replicated.

Per-core algorithm (per local batch b; N=1024, F=O=64, T=12):
  A  = adj * attn_b                      (DVE elementwise, bf16)
  Z1 = A^T X                             (PE bf16; X natural [n,(f,t)])
  Z2 = A^T Z1                            (PE bf16)
  T[j] = transpose_t(X|Z1|Z2)            (PE transposes -> [f,(t,n)] bf16)
  out[n,o,t] = relu(sum_j Theta_j^T T_j) (PE bf16, K=64 accum x3, ACT relu)
"""
import sys

if "/opt/trn_rl_repo" not in sys.path:
    sys.path.insert(0, "/opt/trn_rl_repo")

import numpy as np
from contextlib import ExitStack

import concourse.bass as bass
import concourse.tile as tile
from concourse import bacc, mybir
from concourse.bass_utils import run_bass_kernel_spmd

N_CORES = 8
B, N, F, T, K, O = 16, 1024, 64, 12, 3, 64
BL = B // N_CORES          # local batches per core = 2
NT = N // 128              # n-tiles = 8
FT = F * T                 # 768
OT = O * T                 # 768
XSPLIT = 4                 # x DMA split (2 n-tiles per chunk)

F32 = mybir.dt.float32
BF16 = mybir.dt.bfloat16

_NC = None


class Ctx:
    pass


def _alloc_xz(cx, name):
    """4 tiles of 2 n-tiles each from the shared 8-slot xz tag."""
    tiles = [
        cx.xz_pool.tile([128, 2 * FT], BF16, tag="xz", bufs=8, name=f"{name}_{i}")
        for i in range(XSPLIT)
    ]

    def sl(mt, lo, size):
        return tiles[mt // 2][:, (mt % 2) * FT + lo: (mt % 2) * FT + lo + size]

    return tiles, sl


def _load_x(cx, b):
    tiles, sl = _alloc_xz(cx, f"X{b}")
    for xs in range(XSPLIT):
        cx.nc.gpsimd.dma_start(
            tiles[xs][:],
            cx.x_ap[b, xs * 256:(xs + 1) * 256].rearrange(
                "(nt p) f t -> p nt (f t)", p=128
            ),
        )
    return sl


def _emit_tp_pack(cx, src3, j, nt, g, b, copy_eng=None):
    """One pack: 4 transposes (t = 4g..4g+3) -> psum -> T tile copy."""
    nc = cx.nc
    pt = cx.tp.tile([64, 512], BF16, tag="tp", name="pt")
    for ti in range(4):
        t = 4 * g + ti
        nc.tensor.transpose(
            pt[:, ti * 128:(ti + 1) * 128], src3[:, t, :], cx.ident_t[:]
        )
    if copy_eng is None:
        copy_eng = "v" if (nt + g) % 2 == 0 else "s"
    dst = cx.T_t[(b, j, nt)][:, g * 512:(g + 1) * 512]
    if copy_eng == "v":
        nc.vector.tensor_copy(dst, pt[:])
    else:
        nc.scalar.activation(dst, pt[:], mybir.ActivationFunctionType.Copy)


def _emit_theta_group(cx, b, nt):
    """Theta contraction + relu + out DMA for one n-tile."""
    nc = cx.nc
    o_tile = cx.out_pool.tile([128, OT], F32, tag="out", name="o_tile")
    for (t0, tn) in ((0, 8), (8, 4)):
        pq = cx.qp.tile([128, 512], F32, tag="qp", name="pq")
        for ts in range(tn):
            t = t0 + ts
            for j in range(3):
                nc.tensor.matmul(
                    pq[:, ts * 64:(ts + 1) * 64],
                    cx.T_t[(b, j, nt)][:, t * 128:(t + 1) * 128],
                    cx.theta_t[:, j * 64:(j + 1) * 64],
                    start=(j == 0),
                    stop=(j == 2),
                )
        dst = o_tile[:].rearrange("p (o t) -> p t o", t=T)[:, t0:t0 + tn, :]
        src = pq[:, 0:tn * 64].rearrange("p (t o) -> p t o", o=64)
        nc.scalar.activation(dst, src, mybir.ActivationFunctionType.Relu)
    nc.sync.dma_start(
        cx.out_ap[b, nt * 128:(nt + 1) * 128, :, :].rearrange("p o t -> p (o t)"),
        o_tile[:],
    )


def _emit_A(cx, b, load_adj=False):
    """A(b) = adj * attn[b] as 8 per-mt bf16 tiles (partial-A startup)."""
    nc = cx.nc
    A_t = []
    for mt in range(NT):
        if load_adj:
            adj_t = cx.const_pool.tile(
                [128, 1024], BF16, tag=f"adj{mt}", name=f"adj{mt}"
            )
            nc.gpsimd.dma_start(
                adj_t[:], cx.adj_ap[mt * 128:(mt + 1) * 128, :]
            )
            cx.adj_t.append(adj_t)
        attn_s = cx.scr_pool.tile(
            [128, 1024], BF16, tag="attnscr", bufs=3, name=f"at{b}_{mt}"
        )
        nc.gpsimd.dma_start(
            attn_s[:], cx.attn_ap[b, mt * 128:(mt + 1) * 128, :]
        )
        a = cx.a_pool.tile([128, 1024], BF16, tag=f"A{mt}", bufs=2, name=f"A{b}_{mt}")
        nc.vector.tensor_mul(a[:], attn_s[:], cx.adj_t[mt][:])
        A_t.append(a)
    return A_t


def _emit_big_matmul(cx, A_t, dst_fn, rhs_fn):
    """dst = A^T rhs (16 psum groups of 8 accumulating MMs)."""
    nc = cx.nc
    for nt in range(NT):
        for ch in range(2):
            pz = cx.zp.tile([128, 384], F32, tag="zp", name="pz")
            for mt in range(NT):
                nc.tensor.matmul(
                    pz[:],
                    A_t[mt][:, nt * 128:(nt + 1) * 128],
                    rhs_fn(mt, ch * 384, 384),
                    start=(mt == 0),
                    stop=(mt == NT - 1),
                )
            nc.vector.tensor_copy(dst_fn(nt, ch * 384, 384), pz[:])


def _emit_batch_main(cx, b, x_sl):
    A_t = _emit_A(cx, b, load_adj=(b == 0))

    Z1 = cx.z_pool.tile([128, NT * FT], BF16, tag="Z1", name=f"Z1_{b}")

    def z1_sl(mt, lo, sz):
        return Z1[:, mt * FT + lo: mt * FT + lo + sz]

    _emit_big_matmul(cx, A_t, z1_sl, x_sl)

    # Z1 transposes (contiguous block)
    for nt in range(NT):
        src3 = Z1[:, nt * FT:(nt + 1) * FT].rearrange("p (f t) -> p t f", t=T)
        for g in range(3):
            _emit_tp_pack(cx, src3, 1, nt, g, b)

    _, z2_sl = _alloc_xz(cx, f"Z2_{b}")
    _emit_big_matmul(cx, A_t, z2_sl, z1_sl)

    for nt in range(NT):
        src3 = z2_sl(nt, 0, FT).rearrange("p (f t) -> p t f", t=T)
        for g in range(3):
            _emit_tp_pack(cx, src3, 2, nt, g, b)
        if nt > 0:
            _emit_theta_group(cx, b, nt - 1)
    _emit_theta_group(cx, b, NT - 1)


def _build():
    nc = bacc.Bacc("TRN2", target_bir_lowering=False, debug=False)
    cx = Ctx()
    cx.nc = nc
    cx.x_ap = nc.dram_tensor("x", [BL, N, F, T], F32, kind="ExternalInput").ap()
    cx.attn_ap = nc.dram_tensor(
        "spatial_attention", [BL, N, N], F32, kind="ExternalInput"
    ).ap()
    cx.adj_ap = nc.dram_tensor("adj", [N, N], F32, kind="ExternalInput").ap()
    cx.theta_ap = nc.dram_tensor("Theta", [K, F, O], F32, kind="ExternalInput").ap()
    cx.ident_ap = nc.dram_tensor("ident", [128, 128], F32, kind="ExternalInput").ap()
    cx.out_ap = nc.dram_tensor("out", [BL, N, O, T], F32, kind="ExternalOutput").ap()

    with tile.TileContext(nc) as tc, ExitStack() as ctx:
        cx.a_pool = ctx.enter_context(tc.tile_pool(name="apool", bufs=2))
        cx.xz_pool = ctx.enter_context(tc.tile_pool(name="xz", bufs=1))
        cx.z_pool = ctx.enter_context(tc.tile_pool(name="zpool", bufs=1))
        cx.t_pool = ctx.enter_context(tc.tile_pool(name="tpool", bufs=1))
        cx.scr_pool = ctx.enter_context(tc.tile_pool(name="scr", bufs=3))
        cx.out_pool = ctx.enter_context(tc.tile_pool(name="outp", bufs=3))
        cx.const_pool = ctx.enter_context(tc.tile_pool(name="const", bufs=1))
        cx.zp = ctx.enter_context(tc.tile_pool(name="zp", bufs=3, space="PSUM"))
        cx.tp = ctx.enter_context(tc.tile_pool(name="tp", bufs=3, space="PSUM"))
        cx.qp = ctx.enter_context(tc.tile_pool(name="qp", bufs=2, space="PSUM"))

        cx.ident_t = cx.const_pool.tile([128, 128], BF16, tag="ident")
        nc.gpsimd.dma_start(cx.ident_t[:], cx.ident_ap[:])
        cx.theta_t = cx.const_pool.tile([64, K * O], BF16, tag="theta")
        nc.gpsimd.dma_start(cx.theta_t[:], cx.theta_ap.rearrange("k f o -> f k o"))
        cx.adj_t = []

        # T tiles: (batch, j, nt) -> [64, T*128]; X_T (j=0) double-buffered
        # across batches, j=1,2 single-buffered by shared tag.
        cx.T_t = {}
        for b in range(BL):
            for j in range(3):
                for nt in range(NT):
                    tag = f"T{j}_{nt}" if j > 0 else f"T0_{nt}_{b}"
                    cx.T_t[(b, j, nt)] = cx.t_pool.tile(
                        [64, T * 128], BF16, tag=tag, name=f"T{b}{j}{nt}"
                    )

        # Both batches' x loads + X transposes up-front (PE filler during
        # the attn/adj DMA phase). X-tp copies all go to the Scalar engine
        # so they can't clog DVE ahead of Z1 psum copies.
        x_sls = [_load_x(cx, b) for b in range(BL)]
        for b in range(BL):
            for nt in range(NT):
                src3 = x_sls[b](nt, 0, FT).rearrange("p (f t) -> p t f", t=T)
                for g in range(3):
                    _emit_tp_pack(cx, src3, 0, nt, g, b, copy_eng="s")

        for b in range(BL):
            _emit_batch_main(cx, b, x_sls[b])

    nc.compile()
    return nc


def kernel(**inputs):
    global _NC
    if _NC is None:
        _NC = _build()
    nc = _NC

    x = np.ascontiguousarray(inputs["x"], dtype=np.float32)
    attn = np.ascontiguousarray(inputs["spatial_attention"], dtype=np.float32)
    adj = np.ascontiguousarray(inputs["adj"], dtype=np.float32)
    theta = np.ascontiguousarray(inputs["Theta"], dtype=np.float32)
    ident = np.eye(128, dtype=np.float32)

    in_maps = []
    for i in range(N_CORES):
        s = slice(i * BL, (i + 1) * BL)
        in_maps.append(
            {
                "x": x[s],
                "spatial_attention": attn[s],
                "adj": adj,
                "Theta": theta,
                "ident": ident,
            }
        )
    res = run_bass_kernel_spmd(nc, in_maps, core_ids=list(range(N_CORES)))
    out = np.concatenate([res.results[i]["out"] for i in range(N_CORES)], axis=0)
    return out
